# revision 21
# baseline (speedup 1.0000x reference)
"""Single-head causal self-attention (B=8, T=2048, D=512, H=64), data-parallel
over batch across 8 NeuronCores. Self-contained: builds a Bass/Tile kernel and
runs it via run_bass_kernel_spmd.

Per-core layout (batch element b = core id), bf16 datapath:
  - x is cast to bf16 on the host and DMA-transposed (XBAR) straight into
    SBUF as xT [128, 4, 2048] (d on partitions) -- no PE transposes at all
  - k and q are projected in ONE matmul chain per 512-wide t-chunk
    (lhsT = [Wk | Wq*scale] packed [128, 128]); the k bias is dropped
    (softmax over j is invariant to per-row shifts) and bq is added by the
    PSUM->SBUF copy; kqT [128, T] bf16 holds k on partitions 0:64, q on 64:128
  - v is projected directly in row layout ([t, h], ap=64 matmuls) into
    v_aug [128, 16, 65] whose last column is ones so the PV matmul also
    accumulates the softmax denominator
  - attention per 512-wide i-block in S^T layout: S^T = k_tile^T @ q,
    exp on ACT (pairs of j-tiles), triangular boundary masks via
    affine_select on Pool; the two upper diagonal tiles are computed on
    [256:512] only
  - PV uses out[i, h] = e2_tile^T @ v_aug (ap=65) accumulating all four
    128-wide i-subtiles into a single PSUM bank, which yields the output
    already in row layout: no output transposes
  - epilogue: reciprocal of the denominator column + per-subtile scale on
    DVE/Pool; bv is added on the host (softmax rows sum to 1)
"""

import sys

for _p in ("/root/.axon_site/_ro/trn_rl_repo", "/opt/trn_rl_repo"):
    if _p not in sys.path:
        sys.path.append(_p)

import numpy as np
import ml_dtypes
import concourse.bass as bass
import concourse.bacc as bacc
import concourse.tile as tile
from concourse import mybir
from concourse.bass_utils import run_bass_kernel_spmd

F32 = mybir.dt.float32
BF16 = mybir.dt.bfloat16
EXP = mybir.ActivationFunctionType.Exp

B, T, D, H = 8, 2048, 512, 64
ND = D // 128   # 4 d-chunks
NT = T // 128   # 16 j-tiles
NIB = T // 512  # 4 i-blocks


def build_body(nc, tc, ctx, dram, repeat=1):
    x_d, w_d, bq_d, out_d = dram

    persist = ctx.enter_context(tc.tile_pool(name="persist", bufs=1))
    epool = ctx.enter_context(tc.tile_pool(name="epool", bufs=4))
    rpool = ctx.enter_context(tc.tile_pool(name="rpool", bufs=2))
    psKQ = ctx.enter_context(tc.tile_pool(name="psKQ", bufs=1, space="PSUM"))
    psV = ctx.enter_context(tc.tile_pool(name="psV", bufs=1, space="PSUM"))
    psS = ctx.enter_context(tc.tile_pool(name="psS", bufs=2, space="PSUM"))
    psO = ctx.enter_context(tc.tile_pool(name="psO", bufs=2, space="PSUM"))

    w_sb = persist.tile([128, ND, 3 * 64], BF16)
    bqv = persist.tile([64, 1], F32)
    xT = persist.tile([128, 4, ND, 512], BF16)  # [d%128][tchunk][d//128][t%512]
    kT = persist.tile([64, T], BF16)
    qT = persist.tile([64, T], BF16)
    v_aug = persist.tile([128, NT, 65], BF16)  # v rows + ones column
    o_all = persist.tile([128, NT, 64], BF16)
    warm_in = persist.tile([128, 1], F32)
    warm_out = persist.tile([128, 1], F32)

    # weights/bias via SWDGE (Pool) so they stay off the serial HWDGE
    # completion chain that the x transposes ride on
    nc.gpsimd.dma_start(w_sb[:], w_d[:])
    nc.gpsimd.dma_start(bqv[:], bq_d[:])

    # hoist the exp table load to t~0 (overlaps the input DMAs)
    nc.vector.memset(warm_in[:], 0.0)
    nc.scalar.activation(warm_out[:], warm_in[:], EXP)

    def proj(tch):
        tsl = slice(512 * tch, 512 * (tch + 1))
        kq = psKQ.tile([128, 512], F32, tag="kq")
        for dc in range(ND):
            nc.tensor.matmul(kq[:], w_sb[:, dc, 0:128], xT[:, tch, dc, :],
                             start=(dc == 0), stop=(dc == ND - 1))
        nc.vector.tensor_copy(kT[:, tsl], kq[0:64, :])
        nc.vector.tensor_scalar_add(qT[:, tsl], kq[64:128, :], bqv[:])

        vp = psV.tile([128, 4, 64], F32, tag="v")
        for j in range(4):
            jt = 4 * tch + j
            for dc in range(ND):
                nc.tensor.matmul(vp[:, j, :], xT[:, tch, dc, 128 * j:128 * (j + 1)],
                                 w_sb[:, dc, 128:192],
                                 start=(j == 0 and dc == 0),
                                 stop=(j == 3 and dc == ND - 1),
                                 skip_group_check=True)
        nc.vector.tensor_copy(v_aug[:, 4 * tch:4 * tch + 4, 0:64], vp[:])

    def attn(bi, embed=None, diag_first=False):
        i0 = 512 * bi
        o_ps = psO.tile([128, 4, 65], F32, tag="o")
        # pair descriptors: (jtA, jtB, col0, width) -- i-columns [col0, col0+w)
        full = [(2 * p, 2 * p + 1, 0, 512) for p in range(2 * bi)]
        diag = [(4 * bi, 4 * bi + 1, 0, 512),        # diagonal c=0,1
                (4 * bi + 2, 4 * bi + 3, 256, 256)]  # diagonal c=2,3
        pairs = diag + full if diag_first else full + diag
        n_pv = sum(1 for (jA, jB, _, _) in pairs for jt in (jA, jB)
                   for s in range(4) if jt - 4 * bi < 0 or s >= jt - 4 * bi)
        pv_idx = 0

        def emit_pv(e2, desc):
            nonlocal pv_idx
            jtA, jtB, col0, width = desc
            for h, jt in enumerate((jtA, jtB)):
                c = jt - 4 * bi
                for s in range(4):
                    if c >= 0 and s < c:
                        continue
                    ls = 128 * s - col0
                    nc.tensor.matmul(o_ps[:, s, :], e2[:, h, ls:ls + 128],
                                     v_aug[:, jt, :],
                                     start=(pv_idx == 0),
                                     stop=(pv_idx == n_pv - 1),
                                     skip_group_check=True)
                    pv_idx += 1

        prev = None
        for pi, desc in enumerate(pairs):
            jtA, jtB, col0, width = desc
            st = psS.tile([128, 2, 512], F32, tag="s")
            for h, jt in enumerate((jtA, jtB)):
                nc.tensor.matmul(st[:, h, 0:width],
                                 kT[:, 128 * jt:128 * (jt + 1)],
                                 qT[:, i0 + col0:i0 + col0 + width],
                                 start=True, stop=True)
            e2 = epool.tile([128, 2, 512], BF16, tag="e")
            nc.scalar.activation(e2[:, :, 0:width], st[:, :, 0:width], EXP)
            for h, jt in enumerate((jtA, jtB)):
                c = jt - 4 * bi
                if c >= 0:  # zero the strict upper triangle of the diag tile
                    lo = 128 * c - col0
                    nc.gpsimd.affine_select(
                        out=e2[:, h, lo:lo + 128], in_=e2[:, h, lo:lo + 128],
                        compare_op=mybir.AluOpType.is_ge, fill=0.0,
                        base=0, pattern=[[1, 128]], channel_multiplier=-1)
            if pi == min(1, len(pairs) - 1) and embed is not None:
                embed()  # next t-chunk projections, while ACT chews early exps
            if prev is not None:
                emit_pv(*prev)
            prev = (e2, desc)
        emit_pv(*prev)

        rec = rpool.tile([128, 4, 1], F32, tag="r")
        nc.vector.reciprocal(rec[:], o_ps[:, :, 64:65])
        for s in range(4):
            it = 4 * bi + s
            nc.vector.tensor_scalar_mul(o_all[:, it, :], o_ps[:, s, 0:64],
                                        rec[:, s, :])
        nc.sync.dma_start(
            out_d[i0:i0 + 512, :].rearrange("(a p) h -> p a h", p=128),
            o_all[:, 4 * bi:4 * bi + 4, :])

    for rep in range(repeat):
        nc.vector.memset(v_aug[:, :, 64:65], 1.0)
        # XBAR-transposed x loads (out[p, c, t] = x[t, 128c + p]); the tile
        # scheduler serializes HWDGE DMAs end-to-end (+~2.2us per link)
        for tch in range(4):
            nc.sync.dma_start_transpose(
                xT[:, tch, :, :], x_d[512 * tch:512 * (tch + 1), :])

        proj(0)
        attn(0, embed=lambda: proj(1))
        attn(1, embed=lambda: proj(2))
        attn(2, embed=lambda: proj(3))
        attn(3, diag_first=True)


def build_nc(repeat=1):
    nc = bacc.Bacc("TRN2", target_bir_lowering=False, debug=False, num_devices=8)
    x_d = nc.dram_tensor("x", [T, D], BF16, kind="ExternalInput")
    w_d = nc.dram_tensor("w", [128, ND, 3 * 64], BF16, kind="ExternalInput")
    bq_d = nc.dram_tensor("bq", [64, 1], F32, kind="ExternalInput")
    out_d = nc.dram_tensor("out", [T, H], BF16, kind="ExternalOutput")
    dram = (x_d, w_d, bq_d, out_d)

    from contextlib import ExitStack
    with tile.TileContext(nc) as tc:
        with ExitStack() as ctx:
            build_body(nc, tc, ctx, dram, repeat=repeat)
    nc.compile()
    return nc


_NC_CACHE = {}


def _get_nc(repeat=1):
    if repeat not in _NC_CACHE:
        _NC_CACHE[repeat] = build_nc(repeat)
    return _NC_CACHE[repeat]


def make_in_maps(x, Wk, bk, Wq, bq, Wv, bv):
    scale = float(H) ** -0.5
    bf = ml_dtypes.bfloat16
    # lhsT per d-chunk: columns [Wk | Wq*scale | Wv]; bk is dropped (softmax
    # over j is invariant to shifts that are constant per query row)
    w = np.concatenate(
        [Wk.reshape(ND, 128, H), (Wq * scale).reshape(ND, 128, H),
         Wv.reshape(ND, 128, H)], axis=2)       # [ND, 128, 192]
    w = np.ascontiguousarray(w.transpose(1, 0, 2)).astype(bf)  # [128, ND, 192]
    bqv = np.ascontiguousarray((bq * scale).reshape(64, 1)).astype(np.float32)
    xs = [np.ascontiguousarray(x[b]).astype(bf) for b in range(B)]
    return [{"x": xs[b], "w": w, "bq": bqv} for b in range(B)]


def kernel(x, Wk, bk, Wq, bq, Wv, bv, _repeat=1):
    x = np.asarray(x, dtype=np.float32)
    Wk = np.asarray(Wk, dtype=np.float32)
    bk = np.asarray(bk, dtype=np.float32)
    Wq = np.asarray(Wq, dtype=np.float32)
    bq = np.asarray(bq, dtype=np.float32)
    Wv = np.asarray(Wv, dtype=np.float32)
    bv = np.asarray(bv, dtype=np.float32)

    nc = _get_nc(_repeat)
    in_maps = make_in_maps(x, Wk, bk, Wq, bq, Wv, bv)
    res = run_bass_kernel_spmd(nc, in_maps, core_ids=list(range(B)))
    out = np.stack([np.asarray(res.results[b]["out"]).astype(np.float32)
                    for b in range(B)], axis=0)
    return out + bv[None, None, :]


# revision 22
# speedup vs baseline: 1.0358x; 1.0358x over previous
"""Single-head causal self-attention (B=8, T=2048, D=512, H=64), data-parallel
over batch across 8 NeuronCores. Self-contained: builds a Bass/Tile kernel and
runs it via run_bass_kernel_spmd.

Per-core layout (batch element b = core id), bf16 datapath:
  - x is cast to bf16 on the host and DMA-transposed (XBAR) straight into
    SBUF as xT [128, 4, 2048] (d on partitions) -- no PE transposes at all
  - k and q are projected in ONE matmul chain per 512-wide t-chunk
    (lhsT = [Wk | Wq*scale] packed [128, 128]); the k bias is dropped
    (softmax over j is invariant to per-row shifts) and bq is added by the
    PSUM->SBUF copy; kqT [128, T] bf16 holds k on partitions 0:64, q on 64:128
  - v is projected directly in row layout ([t, h], ap=64 matmuls) into
    v_aug [128, 16, 65] whose last column is ones so the PV matmul also
    accumulates the softmax denominator
  - attention per 512-wide i-block in S^T layout: S^T = k_tile^T @ q,
    exp on ACT (pairs of j-tiles), triangular boundary masks via
    affine_select on Pool; the two upper diagonal tiles are computed on
    [256:512] only
  - PV uses out[i, h] = e2_tile^T @ v_aug (ap=65) accumulating all four
    128-wide i-subtiles into a single PSUM bank, which yields the output
    already in row layout: no output transposes
  - epilogue: reciprocal of the denominator column + per-subtile scale on
    DVE/Pool; bv is added on the host (softmax rows sum to 1)
"""

import sys

for _p in ("/root/.axon_site/_ro/trn_rl_repo", "/opt/trn_rl_repo"):
    if _p not in sys.path:
        sys.path.append(_p)

import numpy as np
import ml_dtypes
import concourse.bass as bass
import concourse.bacc as bacc
import concourse.tile as tile
from concourse import mybir
from concourse.bass_utils import run_bass_kernel_spmd

F32 = mybir.dt.float32
BF16 = mybir.dt.bfloat16
EXP = mybir.ActivationFunctionType.Exp

B, T, D, H = 8, 2048, 512, 64
ND = D // 128   # 4 d-chunks
NT = T // 128   # 16 j-tiles
NIB = T // 512  # 4 i-blocks


def build_body(nc, tc, ctx, dram, repeat=1):
    x_d, w_d, bq_d, out_d = dram

    persist = ctx.enter_context(tc.tile_pool(name="persist", bufs=1))
    epool = ctx.enter_context(tc.tile_pool(name="epool", bufs=4))
    rpool = ctx.enter_context(tc.tile_pool(name="rpool", bufs=2))
    psKQ = ctx.enter_context(tc.tile_pool(name="psKQ", bufs=1, space="PSUM"))
    psV = ctx.enter_context(tc.tile_pool(name="psV", bufs=1, space="PSUM"))
    psS = ctx.enter_context(tc.tile_pool(name="psS", bufs=2, space="PSUM"))
    psO = ctx.enter_context(tc.tile_pool(name="psO", bufs=2, space="PSUM"))

    w_sb = persist.tile([128, ND, 3 * 64], BF16)
    bqv = persist.tile([64, 1], F32)
    xT = persist.tile([128, 4, ND, 512], BF16)  # [d%128][tchunk][d//128][t%512]
    kT = persist.tile([64, T], BF16)
    qT = persist.tile([64, T], BF16)
    v_aug = persist.tile([128, NT, 65], BF16)  # v rows + ones column
    o_all = persist.tile([128, NT, 64], BF16)
    warm_in = persist.tile([128, 1], F32)
    warm_out = persist.tile([128, 1], F32)

    # ALL DMAs ride the SP queue: the tile scheduler cross-chains DMAs that
    # sit on different queues (+~2.2us per link) but leaves same-queue DMAs
    # free to pipeline on the DMA engines
    nc.sync.dma_start(w_sb[:], w_d[:])
    nc.sync.dma_start(bqv[:], bq_d[:])

    # hoist the exp table load to t~0 (overlaps the input DMAs)
    nc.vector.memset(warm_in[:], 0.0)
    nc.scalar.activation(warm_out[:], warm_in[:], EXP)

    def proj(tch):
        tsl = slice(512 * tch, 512 * (tch + 1))
        kq = psKQ.tile([128, 512], F32, tag="kq")
        for dc in range(ND):
            nc.tensor.matmul(kq[:], w_sb[:, dc, 0:128], xT[:, tch, dc, :],
                             start=(dc == 0), stop=(dc == ND - 1))
        nc.vector.tensor_copy(kT[:, tsl], kq[0:64, :])
        nc.vector.tensor_scalar_add(qT[:, tsl], kq[64:128, :], bqv[:])

        vp = psV.tile([128, 4, 64], F32, tag="v")
        for j in range(4):
            jt = 4 * tch + j
            for dc in range(ND):
                nc.tensor.matmul(vp[:, j, :], xT[:, tch, dc, 128 * j:128 * (j + 1)],
                                 w_sb[:, dc, 128:192],
                                 start=(j == 0 and dc == 0),
                                 stop=(j == 3 and dc == ND - 1),
                                 skip_group_check=True)
        nc.vector.tensor_copy(v_aug[:, 4 * tch:4 * tch + 4, 0:64], vp[:])

    def attn(bi, embed=None, diag_first=False):
        i0 = 512 * bi
        o_ps = psO.tile([128, 4, 65], F32, tag="o")
        # pair descriptors: (jtA, jtB, col0, width) -- i-columns [col0, col0+w)
        full = [(2 * p, 2 * p + 1, 0, 512) for p in range(2 * bi)]
        diag = [(4 * bi, 4 * bi + 1, 0, 512),        # diagonal c=0,1
                (4 * bi + 2, 4 * bi + 3, 256, 256)]  # diagonal c=2,3
        pairs = diag + full if diag_first else full + diag
        n_pv = sum(1 for (jA, jB, _, _) in pairs for jt in (jA, jB)
                   for s in range(4) if jt - 4 * bi < 0 or s >= jt - 4 * bi)
        pv_idx = 0

        def emit_pv(e2, desc):
            nonlocal pv_idx
            jtA, jtB, col0, width = desc
            for h, jt in enumerate((jtA, jtB)):
                c = jt - 4 * bi
                for s in range(4):
                    if c >= 0 and s < c:
                        continue
                    ls = 128 * s - col0
                    nc.tensor.matmul(o_ps[:, s, :], e2[:, h, ls:ls + 128],
                                     v_aug[:, jt, :],
                                     start=(pv_idx == 0),
                                     stop=(pv_idx == n_pv - 1),
                                     skip_group_check=True)
                    pv_idx += 1

        prev = None
        for pi, desc in enumerate(pairs):
            jtA, jtB, col0, width = desc
            st = psS.tile([128, 2, 512], F32, tag="s")
            for h, jt in enumerate((jtA, jtB)):
                nc.tensor.matmul(st[:, h, 0:width],
                                 kT[:, 128 * jt:128 * (jt + 1)],
                                 qT[:, i0 + col0:i0 + col0 + width],
                                 start=True, stop=True)
            e2 = epool.tile([128, 2, 512], BF16, tag="e")
            nc.scalar.activation(e2[:, :, 0:width], st[:, :, 0:width], EXP)
            for h, jt in enumerate((jtA, jtB)):
                c = jt - 4 * bi
                if c >= 0:  # zero the strict upper triangle of the diag tile
                    lo = 128 * c - col0
                    nc.gpsimd.affine_select(
                        out=e2[:, h, lo:lo + 128], in_=e2[:, h, lo:lo + 128],
                        compare_op=mybir.AluOpType.is_ge, fill=0.0,
                        base=0, pattern=[[1, 128]], channel_multiplier=-1)
            if pi == min(1, len(pairs) - 1) and embed is not None:
                embed()  # next t-chunk projections, while ACT chews early exps
            if prev is not None:
                emit_pv(*prev)
            prev = (e2, desc)
        emit_pv(*prev)

        rec = rpool.tile([128, 4, 1], F32, tag="r")
        nc.vector.reciprocal(rec[:], o_ps[:, :, 64:65])
        for s in range(4):
            it = 4 * bi + s
            nc.vector.tensor_scalar_mul(o_all[:, it, :], o_ps[:, s, 0:64],
                                        rec[:, s, :])
        nc.sync.dma_start(
            out_d[i0:i0 + 512, :].rearrange("(a p) h -> p a h", p=128),
            o_all[:, 4 * bi:4 * bi + 4, :])

    for rep in range(repeat):
        nc.vector.memset(v_aug[:, :, 64:65], 1.0)
        # XBAR-transposed x loads (out[p, c, t] = x[t, 128c + p]); the tile
        # scheduler serializes HWDGE DMAs end-to-end (+~2.2us per link)
        for tch in range(4):
            nc.sync.dma_start_transpose(
                xT[:, tch, :, :], x_d[512 * tch:512 * (tch + 1), :])

        proj(0)
        attn(0, embed=lambda: proj(1))
        attn(1, embed=lambda: proj(2))
        attn(2, embed=lambda: proj(3))
        attn(3, diag_first=True)


def build_nc(repeat=1):
    nc = bacc.Bacc("TRN2", target_bir_lowering=False, debug=False, num_devices=8)
    x_d = nc.dram_tensor("x", [T, D], BF16, kind="ExternalInput")
    w_d = nc.dram_tensor("w", [128, ND, 3 * 64], BF16, kind="ExternalInput")
    bq_d = nc.dram_tensor("bq", [64, 1], F32, kind="ExternalInput")
    out_d = nc.dram_tensor("out", [T, H], BF16, kind="ExternalOutput")
    dram = (x_d, w_d, bq_d, out_d)

    from contextlib import ExitStack
    with tile.TileContext(nc) as tc:
        with ExitStack() as ctx:
            build_body(nc, tc, ctx, dram, repeat=repeat)
    nc.compile()
    return nc


_NC_CACHE = {}


def _get_nc(repeat=1):
    if repeat not in _NC_CACHE:
        _NC_CACHE[repeat] = build_nc(repeat)
    return _NC_CACHE[repeat]


def make_in_maps(x, Wk, bk, Wq, bq, Wv, bv):
    scale = float(H) ** -0.5
    bf = ml_dtypes.bfloat16
    # lhsT per d-chunk: columns [Wk | Wq*scale | Wv]; bk is dropped (softmax
    # over j is invariant to shifts that are constant per query row)
    w = np.concatenate(
        [Wk.reshape(ND, 128, H), (Wq * scale).reshape(ND, 128, H),
         Wv.reshape(ND, 128, H)], axis=2)       # [ND, 128, 192]
    w = np.ascontiguousarray(w.transpose(1, 0, 2)).astype(bf)  # [128, ND, 192]
    bqv = np.ascontiguousarray((bq * scale).reshape(64, 1)).astype(np.float32)
    xs = [np.ascontiguousarray(x[b]).astype(bf) for b in range(B)]
    return [{"x": xs[b], "w": w, "bq": bqv} for b in range(B)]


def kernel(x, Wk, bk, Wq, bq, Wv, bv, _repeat=1):
    x = np.asarray(x, dtype=np.float32)
    Wk = np.asarray(Wk, dtype=np.float32)
    bk = np.asarray(bk, dtype=np.float32)
    Wq = np.asarray(Wq, dtype=np.float32)
    bq = np.asarray(bq, dtype=np.float32)
    Wv = np.asarray(Wv, dtype=np.float32)
    bv = np.asarray(bv, dtype=np.float32)

    nc = _get_nc(_repeat)
    in_maps = make_in_maps(x, Wk, bk, Wq, bq, Wv, bv)
    res = run_bass_kernel_spmd(nc, in_maps, core_ids=list(range(B)))
    out = np.stack([np.asarray(res.results[b]["out"]).astype(np.float32)
                    for b in range(B)], axis=0)
    return out + bv[None, None, :]


# revision 27
# speedup vs baseline: 1.0945x; 1.0567x over previous
"""Single-head causal self-attention (B=8, T=2048, D=512, H=64), data-parallel
over batch across 8 NeuronCores. Self-contained: builds a Bass/Tile kernel and
runs it via run_bass_kernel_spmd.

Per-core layout (batch element b = core id), bf16 datapath:
  - the host packs [x ; Wk^T ; Wq^T*s ; Wv^T ; bq*s] into one bf16 DRAM
    tensor xw [2256, 512]; XBAR DMA-transposes deliver both xT (d on
    partitions) and the weight block w_sb in matmul-ready layout -- no PE
    transposes, no separate weight DMAs (mixed DMACopy/transpose prologues
    get serialized by the tile scheduler's cross-DMA chaining)
  - k and q are projected in ONE matmul chain per 512-wide t-chunk
    (lhsT = [Wk | Wq*s] packed [128, 128]); the k bias is dropped (softmax
    over j is invariant to per-row shifts) and bq is added by the
    PSUM->SBUF copy (kT/qT split tiles, cross-partition-base DVE copies)
  - v is projected directly in row layout ([t, h], ap=64 matmuls) into
    v_aug [128, 16, 65] whose last column is ones so the PV matmul also
    accumulates the softmax denominator
  - attention per 512-wide i-block in S^T layout: S^T = k_tile^T @ q, exp
    on ACT (pairs of j-tiles), triangular boundary masks via affine_select
    on Pool; the upper two diagonal tiles are computed on [256:512] only
  - PV uses out[i, h] = e2_tile^T @ v_aug (ap=65), accumulating all four
    128-wide i-subtiles of a block in a single PSUM bank -> output lands
    in row layout, no output transposes
  - dummy PE matmuls at t=0 ramp the tensor engine to full clock while the
    x DMAs are in flight; exp table load is also hoisted to t=0
  - epilogue: reciprocal of the denominator column + per-subtile scale on
    DVE; bv is added on the host (softmax rows sum to 1)
"""

import sys

for _p in ("/root/.axon_site/_ro/trn_rl_repo", "/opt/trn_rl_repo"):
    if _p not in sys.path:
        sys.path.append(_p)

import numpy as np
import ml_dtypes
import concourse.bass as bass
import concourse.bacc as bacc
import concourse.tile as tile
from concourse import mybir
from concourse.bass_utils import run_bass_kernel_spmd

F32 = mybir.dt.float32
BF16 = mybir.dt.bfloat16
EXP = mybir.ActivationFunctionType.Exp

B, T, D, H = 8, 2048, 512, 64
ND = D // 128   # 4 d-chunks
NT = T // 128   # 16 j-tiles
NIB = T // 512  # 4 i-blocks
WROWS = 208     # 192 weight columns + bias row + pad to multiple of 16
N_WARM = 12     # dummy matmuls covering the ~3us PE p-state ramp


def build_body(nc, tc, ctx, dram, repeat=1):
    x_d, out_d = dram

    persist = ctx.enter_context(tc.tile_pool(name="persist", bufs=1))
    epool = ctx.enter_context(tc.tile_pool(name="epool", bufs=4))
    rpool = ctx.enter_context(tc.tile_pool(name="rpool", bufs=2))
    psKQ = ctx.enter_context(tc.tile_pool(name="psKQ", bufs=1, space="PSUM"))
    psV = ctx.enter_context(tc.tile_pool(name="psV", bufs=1, space="PSUM"))
    psS = ctx.enter_context(tc.tile_pool(name="psS", bufs=2, space="PSUM"))
    psO = ctx.enter_context(tc.tile_pool(name="psO", bufs=2, space="PSUM"))

    w_sb = persist.tile([128, ND, WROWS], BF16)
    xT = persist.tile([128, 4, ND, 512], BF16)  # [d%128][tchunk][d//128][t%512]
    kT = persist.tile([64, T], BF16)
    qT = persist.tile([64, T], BF16)
    v_aug = persist.tile([128, NT, 65], BF16)  # v rows + ones column
    o_all = persist.tile([128, NT, 64], BF16)
    warm_mm = persist.tile([128, 512], BF16)
    warm_out = persist.tile([128, 1], BF16)

    # hoist the exp table load to t~0 and ramp the PE to full clock with
    # dummy matmuls while the input DMAs are in flight
    nc.vector.memset(warm_mm[:], 0.0)
    nc.scalar.activation(warm_out[:], warm_mm[:, 0:1], EXP)
    for _ in range(N_WARM):
        wps = psKQ.tile([128, 512], F32, tag="kq")
        nc.tensor.matmul(wps[:], warm_mm[:, 0:128], warm_mm[:],
                         start=True, stop=True)

    # bq*scale arrives in bf16 via the weight transpose (partitions 64:128);
    # tensor_scalar needs a float32 scalar, so widen it once
    bqv = persist.tile([64, 1], F32)
    bq_ap = bqv[:]

    def proj(tch):
        tsl = slice(512 * tch, 512 * (tch + 1))
        kq = psKQ.tile([128, 512], F32, tag="kq")
        for dc in range(ND):
            nc.tensor.matmul(kq[:], w_sb[:, dc, 0:128], xT[:, tch, dc, :],
                             start=(dc == 0), stop=(dc == ND - 1))
        nc.vector.tensor_copy(kT[:, tsl], kq[0:64, :])
        nc.vector.tensor_scalar_add(qT[:, tsl], kq[64:128, :], bq_ap)

        vp = psV.tile([128, 4, 64], F32, tag="v")
        for j in range(4):
            for dc in range(ND):
                nc.tensor.matmul(vp[:, j, :],
                                 xT[:, tch, dc, 128 * j:128 * (j + 1)],
                                 w_sb[:, dc, 128:192],
                                 start=(j == 0 and dc == 0),
                                 stop=(j == 3 and dc == ND - 1),
                                 skip_group_check=True)
        nc.vector.tensor_copy(v_aug[:, 4 * tch:4 * tch + 4, 0:64], vp[:])

    def attn(bi, embed=None, diag_first=False):
        i0 = 512 * bi
        o_ps = psO.tile([128, 4, 65], F32, tag="o")
        # pair descriptors: (jtA, jtB, col0, width) -- i-columns [col0, col0+w)
        full = [(2 * p, 2 * p + 1, 0, 512) for p in range(2 * bi)]
        diag = [(4 * bi, 4 * bi + 1, 0, 512),        # diagonal c=0,1
                (4 * bi + 2, 4 * bi + 3, 256, 256)]  # diagonal c=2,3
        pairs = diag + full if diag_first else full + diag
        n_pv = sum(1 for (jA, jB, _, _) in pairs for jt in (jA, jB)
                   for s in range(4) if jt - 4 * bi < 0 or s >= jt - 4 * bi)
        pv_idx = 0

        def emit_pv(e2, desc):
            nonlocal pv_idx
            jtA, jtB, col0, width = desc
            for h, jt in enumerate((jtA, jtB)):
                c = jt - 4 * bi
                for s in range(4):
                    if c >= 0 and s < c:
                        continue
                    ls = 128 * s - col0
                    nc.tensor.matmul(o_ps[:, s, :], e2[:, h, ls:ls + 128],
                                     v_aug[:, jt, :],
                                     start=(pv_idx == 0),
                                     stop=(pv_idx == n_pv - 1),
                                     skip_group_check=True)
                    pv_idx += 1

        prev = None
        for pi, desc in enumerate(pairs):
            jtA, jtB, col0, width = desc
            st = psS.tile([128, 2, 512], F32, tag="s")
            for h, jt in enumerate((jtA, jtB)):
                nc.tensor.matmul(st[:, h, 0:width],
                                 kT[:, 128 * jt:128 * (jt + 1)],
                                 qT[:, i0 + col0:i0 + col0 + width],
                                 start=True, stop=True)
            e2 = epool.tile([128, 2, 512], BF16, tag="e")
            nc.scalar.activation(e2[:, :, 0:width], st[:, :, 0:width], EXP)
            for h, jt in enumerate((jtA, jtB)):
                c = jt - 4 * bi
                if c >= 0:  # zero the strict upper triangle of the diag tile
                    lo = 128 * c - col0
                    nc.gpsimd.affine_select(
                        out=e2[:, h, lo:lo + 128], in_=e2[:, h, lo:lo + 128],
                        compare_op=mybir.AluOpType.is_ge, fill=0.0,
                        base=0, pattern=[[1, 128]], channel_multiplier=-1)
            if pi == min(1, len(pairs) - 1) and embed is not None:
                embed()  # next t-chunk projections, while ACT chews early exps
            if prev is not None:
                emit_pv(*prev)
            prev = (e2, desc)
        emit_pv(*prev)

        rec = rpool.tile([128, 4, 1], F32, tag="r")
        nc.vector.reciprocal(rec[:], o_ps[:, :, 64:65])
        for s in range(4):
            it = 4 * bi + s
            nc.vector.tensor_scalar_mul(o_all[:, it, :], o_ps[:, s, 0:64],
                                        rec[:, s, :])
        nc.sync.dma_start(
            out_d[i0:i0 + 512, :].rearrange("(a p) h -> p a h", p=128),
            o_all[:, 4 * bi:4 * bi + 4, :])

    for rep in range(repeat):
        nc.vector.memset(v_aug[:, :, 64:65], 1.0)
        # XBAR-transposed loads (out[p, c, r] = in[r, 128c + p]); keeping the
        # prologue pure-transpose on one queue avoids the tile scheduler's
        # cross-DMA serialization chains
        if rep == 0:
            nc.sync.dma_start_transpose(w_sb[:], x_d[T:T + WROWS, :])
            nc.vector.tensor_copy(bqv[:], w_sb[64:128, 0, 192:193])
        for tch in range(4):
            nc.sync.dma_start_transpose(
                xT[:, tch, :, :], x_d[512 * tch:512 * (tch + 1), :])

        proj(0)
        attn(0, embed=lambda: proj(1))
        attn(1, embed=lambda: proj(2))
        attn(2, embed=lambda: proj(3))
        attn(3, diag_first=True)


def build_nc(repeat=1):
    nc = bacc.Bacc("TRN2", target_bir_lowering=False, debug=False, num_devices=8)
    x_d = nc.dram_tensor("xw", [T + WROWS, D], BF16, kind="ExternalInput")
    out_d = nc.dram_tensor("out", [T, H], BF16, kind="ExternalOutput")

    from contextlib import ExitStack
    with tile.TileContext(nc) as tc:
        with ExitStack() as ctx:
            build_body(nc, tc, ctx, (x_d, out_d), repeat=repeat)
    nc.compile()
    return nc


_NC_CACHE = {}


def _get_nc(repeat=1):
    if repeat not in _NC_CACHE:
        _NC_CACHE[repeat] = build_nc(repeat)
    return _NC_CACHE[repeat]


def make_in_maps(x, Wk, bk, Wq, bq, Wv, bv):
    scale = float(H) ** -0.5
    bf = ml_dtypes.bfloat16
    # weight block rows r, cols d: w_sb[p, dc, r] = WB[r, 128 dc + p]
    #   r 0:64 = Wk^T, 64:128 = (Wq*s)^T, 128:192 = Wv^T,
    #   r 192 cols 64:128 = bq*s (k-bias dropped: softmax shift invariance)
    WB = np.zeros((WROWS, D), np.float32)
    WB[0:64] = Wk.T
    WB[64:128] = (Wq * scale).T
    WB[128:192] = Wv.T
    WB[192, 64:128] = bq * scale
    WB = WB.astype(bf)
    return [{"xw": np.concatenate([np.asarray(x[b]).astype(bf), WB], axis=0)}
            for b in range(B)]


def kernel(x, Wk, bk, Wq, bq, Wv, bv, _repeat=1):
    x = np.asarray(x, dtype=np.float32)
    Wk = np.asarray(Wk, dtype=np.float32)
    bk = np.asarray(bk, dtype=np.float32)
    Wq = np.asarray(Wq, dtype=np.float32)
    bq = np.asarray(bq, dtype=np.float32)
    Wv = np.asarray(Wv, dtype=np.float32)
    bv = np.asarray(bv, dtype=np.float32)

    nc = _get_nc(_repeat)
    in_maps = make_in_maps(x, Wk, bk, Wq, bq, Wv, bv)
    res = run_bass_kernel_spmd(nc, in_maps, core_ids=list(range(B)))
    out = np.stack([np.asarray(res.results[b]["out"]).astype(np.float32)
                    for b in range(B)], axis=0)
    return out + bv[None, None, :]


# revision 37
# speedup vs baseline: 1.1559x; 1.0560x over previous
"""Single-head causal self-attention (B=8, T=2048, D=512, H=64), data-parallel
over batch across 8 NeuronCores. Self-contained: builds a Bass/Tile kernel and
runs it via run_bass_kernel_spmd.

Per-core layout (batch element b = core id), bf16 datapath:
  - the host packs [x ; Wk^T ; Wq^T*s ; Wv^T ; bq*s] into one bf16 DRAM
    tensor xw [2256, 512]; XBAR DMA-transposes deliver both xT (d on
    partitions) and the weight block w_sb in matmul-ready layout -- no PE
    transposes, no separate weight DMAs (mixed DMACopy/transpose prologues
    get serialized by the tile scheduler's cross-DMA chaining)
  - k and q are projected in ONE matmul chain per 512-wide t-chunk
    (lhsT = [Wk | Wq*s] packed [128, 128]); the k bias is dropped (softmax
    over j is invariant to per-row shifts) and bq is added by the
    PSUM->SBUF copy (kT/qT split tiles, cross-partition-base DVE copies)
  - v is projected directly in row layout ([t, h], ap=64 matmuls) into
    v_aug [128, 16, 65] whose last column is ones so the PV matmul also
    accumulates the softmax denominator
  - attention per 512-wide i-block in S^T layout: S^T = k_tile^T @ q, exp
    on ACT (pairs of j-tiles), triangular boundary masks via affine_select
    on Pool; the upper two diagonal tiles are computed on [256:512] only
  - PV uses out[i, h] = e2_tile^T @ v_aug (ap=65), accumulating all four
    128-wide i-subtiles of a block in a single PSUM bank -> output lands
    in row layout, no output transposes
  - dummy PE matmuls at t=0 ramp the tensor engine to full clock while the
    x DMAs are in flight; exp table load is also hoisted to t=0
  - epilogue: reciprocal of the denominator column + per-subtile scale on
    DVE; bv is added on the host (softmax rows sum to 1)
"""

import sys

for _p in ("/root/.axon_site/_ro/trn_rl_repo", "/opt/trn_rl_repo"):
    if _p not in sys.path:
        sys.path.append(_p)

import numpy as np
import ml_dtypes
import concourse.bass as bass
import concourse.bacc as bacc
import concourse.tile as tile
from concourse import mybir
from concourse.bass_utils import run_bass_kernel_spmd

F32 = mybir.dt.float32
BF16 = mybir.dt.bfloat16
EXP = mybir.ActivationFunctionType.Exp

B, T, D, H = 8, 2048, 512, 64
ND = D // 128   # 4 d-chunks
NT = T // 128   # 16 j-tiles
NIB = T // 512  # 4 i-blocks
WROWS = 208     # 192 weight columns + bias row + pad to multiple of 16
N_WARM = 12     # dummy matmuls covering the ~3us PE p-state ramp


def build_body(nc, tc, ctx, dram, repeat=1):
    x_d, out_d = dram

    persist = ctx.enter_context(tc.tile_pool(name="persist", bufs=1))
    epool = ctx.enter_context(tc.tile_pool(name="epool", bufs=4))
    opool = ctx.enter_context(tc.tile_pool(name="opool", bufs=2))
    psKQ = ctx.enter_context(tc.tile_pool(name="psKQ", bufs=1, space="PSUM"))
    psV = ctx.enter_context(tc.tile_pool(name="psV", bufs=1, space="PSUM"))
    psS = ctx.enter_context(tc.tile_pool(name="psS", bufs=2, space="PSUM"))
    psO = ctx.enter_context(tc.tile_pool(name="psO", bufs=2, space="PSUM"))

    w_sb = persist.tile([128, ND, WROWS], BF16)
    xT = persist.tile([128, 4, ND, 512], BF16)  # [d%128][tchunk][d//128][t%512]
    kT = persist.tile([64, T], BF16)
    qT = persist.tile([64, T], BF16)
    v_aug = persist.tile([128, NT, 65], BF16)  # v rows + ones column
    warm_mm = persist.tile([128, 512], BF16)
    warm_out = persist.tile([128, 1], BF16)

    # hoist the exp table load to t~0 and ramp the PE to full clock with
    # dummy matmuls while the input DMAs are in flight
    nc.vector.memset(warm_mm[:], 0.0)
    nc.scalar.activation(warm_out[:], warm_mm[:, 0:1], EXP)
    for _ in range(N_WARM):
        wps = psKQ.tile([128, 512], F32, tag="kq")
        nc.tensor.matmul(wps[:], warm_mm[:, 0:128], warm_mm[:],
                         start=True, stop=True)

    # bq*scale arrives in bf16 via the weight transpose (partitions 64:128);
    # tensor_scalar needs a float32 scalar, so widen it once
    bqv = persist.tile([64, 1], F32)
    bq_ap = bqv[:]

    def proj(tch):
        tsl = slice(512 * tch, 512 * (tch + 1))
        kq = psKQ.tile([128, 512], F32, tag="kq")
        for dc in range(ND):
            nc.tensor.matmul(kq[:], w_sb[:, dc, 0:128], xT[:, tch, dc, :],
                             start=(dc == 0), stop=(dc == ND - 1))
        # qT first: it gates the next i-block's S matmuls (kT is only needed
        # as lhsT for tiles that are ready much earlier)
        nc.vector.tensor_scalar_add(qT[:, tsl], kq[64:128, :], bq_ap)
        nc.vector.tensor_copy(kT[:, tsl], kq[0:64, :])

        vp = psV.tile([128, 4, 64], F32, tag="v")
        for j in range(4):
            for dc in range(ND):
                nc.tensor.matmul(vp[:, j, :],
                                 xT[:, tch, dc, 128 * j:128 * (j + 1)],
                                 w_sb[:, dc, 128:192],
                                 start=(j == 0 and dc == 0),
                                 stop=(j == 3 and dc == ND - 1),
                                 skip_group_check=True)
        nc.vector.tensor_copy(v_aug[:, 4 * tch:4 * tch + 4, 0:64], vp[:])

    def attn(bi, embed=None, diag_first=False):
        i0 = 512 * bi
        o_ps = psO.tile([128, 4, 65], F32, tag="o")
        # pair descriptors: (jtA, jtB, col0, width) -- i-columns [col0, col0+w)
        full = [(2 * p, 2 * p + 1, 0, 512) for p in range(2 * bi)]
        diag = [(4 * bi, 4 * bi + 1, 0, 512),        # diagonal c=0,1
                (4 * bi + 2, 4 * bi + 3, 256, 256)]  # diagonal c=2,3
        pairs = diag + full if diag_first else full + diag
        n_pv = sum(1 for (jA, jB, _, _) in pairs for jt in (jA, jB)
                   for s in range(4) if jt - 4 * bi < 0 or s >= jt - 4 * bi)
        pv_idx = 0

        def emit_pv(e2, desc):
            nonlocal pv_idx
            jtA, jtB, col0, width = desc
            for h, jt in enumerate((jtA, jtB)):
                c = jt - 4 * bi
                for s in range(4):
                    if c >= 0 and s < c:
                        continue
                    ls = 128 * s - col0
                    nc.tensor.matmul(o_ps[:, s, :], e2[:, h, ls:ls + 128],
                                     v_aug[:, jt, :],
                                     start=(pv_idx == 0),
                                     stop=(pv_idx == n_pv - 1),
                                     skip_group_check=True)
                    pv_idx += 1

        prev = None
        for pi, desc in enumerate(pairs):
            jtA, jtB, col0, width = desc
            st = psS.tile([128, 2, 512], F32, tag="s")
            for h, jt in enumerate((jtA, jtB)):
                nc.tensor.matmul(st[:, h, 0:width],
                                 kT[:, 128 * jt:128 * (jt + 1)],
                                 qT[:, i0 + col0:i0 + col0 + width],
                                 start=True, stop=True)
            e2 = epool.tile([128, 2, 512], BF16, tag="e")
            nc.scalar.activation(e2[:, :, 0:width], st[:, :, 0:width], EXP)
            for h, jt in enumerate((jtA, jtB)):
                c = jt - 4 * bi
                if c >= 0:  # zero the strict upper triangle of the diag tile
                    lo = 128 * c - col0
                    nc.gpsimd.affine_select(
                        out=e2[:, h, lo:lo + 128], in_=e2[:, h, lo:lo + 128],
                        compare_op=mybir.AluOpType.is_ge, fill=0.0,
                        base=0, pattern=[[1, 128]], channel_multiplier=-1)
            if pi == min(1, len(pairs) - 1) and embed is not None:
                embed()  # next t-chunk projections, while ACT chews early exps
            if prev is not None:
                emit_pv(*prev)
            prev = (e2, desc)
        emit_pv(*prev)

        # store numerator+denominator raw; the host does the divide
        o_sb = opool.tile([128, 4, 65], BF16, tag="os")
        nc.vector.tensor_copy(o_sb[:], o_ps[:])
        nc.sync.dma_start(
            out_d[i0:i0 + 512, :].rearrange("(a p) h -> p a h", p=128),
            o_sb[:])

    for rep in range(repeat):
        nc.vector.memset(v_aug[:, :, 64:65], 1.0)
        # XBAR-transposed loads (out[p, c, r] = in[r, 128c + p]); keeping the
        # prologue pure-transpose on one queue avoids the tile scheduler's
        # cross-DMA serialization chains
        if rep == 0:
            nc.sync.dma_start_transpose(w_sb[:], x_d[T:T + WROWS, :])
            nc.vector.tensor_copy(bqv[:], w_sb[64:128, 0, 192:193])
        for tch in range(4):
            nc.sync.dma_start_transpose(
                xT[:, tch, :, :], x_d[512 * tch:512 * (tch + 1), :])

        proj(0)
        attn(0, embed=lambda: proj(1))
        attn(1, embed=lambda: proj(2))
        attn(2, embed=lambda: proj(3))
        attn(3, diag_first=True)


def build_nc(repeat=1):
    nc = bacc.Bacc("TRN2", target_bir_lowering=False, debug=False, num_devices=8)
    x_d = nc.dram_tensor("xw", [T + WROWS, D], BF16, kind="ExternalInput")
    out_d = nc.dram_tensor("out", [T, H + 1], BF16, kind="ExternalOutput")

    from contextlib import ExitStack
    with tile.TileContext(nc) as tc:
        with ExitStack() as ctx:
            build_body(nc, tc, ctx, (x_d, out_d), repeat=repeat)
    nc.compile()
    return nc


_NC_CACHE = {}


def _get_nc(repeat=1):
    if repeat not in _NC_CACHE:
        _NC_CACHE[repeat] = build_nc(repeat)
    return _NC_CACHE[repeat]


def make_in_maps(x, Wk, bk, Wq, bq, Wv, bv):
    scale = float(H) ** -0.5
    bf = ml_dtypes.bfloat16
    # weight block rows r, cols d: w_sb[p, dc, r] = WB[r, 128 dc + p]
    #   r 0:64 = Wk^T, 64:128 = (Wq*s)^T, 128:192 = Wv^T,
    #   r 192 cols 64:128 = bq*s (k-bias dropped: softmax shift invariance)
    WB = np.zeros((WROWS, D), np.float32)
    WB[0:64] = Wk.T
    WB[64:128] = (Wq * scale).T
    WB[128:192] = Wv.T
    WB[192, 64:128] = bq * scale
    WB = WB.astype(bf)
    return [{"xw": np.concatenate([np.asarray(x[b]).astype(bf), WB], axis=0)}
            for b in range(B)]


def kernel(x, Wk, bk, Wq, bq, Wv, bv, _repeat=1):
    x = np.asarray(x, dtype=np.float32)
    Wk = np.asarray(Wk, dtype=np.float32)
    bk = np.asarray(bk, dtype=np.float32)
    Wq = np.asarray(Wq, dtype=np.float32)
    bq = np.asarray(bq, dtype=np.float32)
    Wv = np.asarray(Wv, dtype=np.float32)
    bv = np.asarray(bv, dtype=np.float32)

    nc = _get_nc(_repeat)
    in_maps = make_in_maps(x, Wk, bk, Wq, bq, Wv, bv)
    res = run_bass_kernel_spmd(nc, in_maps, core_ids=list(range(B)))
    raw = np.stack([np.asarray(res.results[b]["out"]).astype(np.float32)
                    for b in range(B)], axis=0)
    return raw[..., 0:64] / raw[..., 64:65] + bv[None, None, :]


# revision 39
# speedup vs baseline: 1.2165x; 1.0524x over previous
"""Single-head causal self-attention (B=8, T=2048, D=512, H=64), data-parallel
over batch across 8 NeuronCores. Self-contained: builds a Bass/Tile kernel and
runs it via run_bass_kernel_spmd.

Per-core layout (batch element b = core id), bf16 datapath:
  - the host packs [x ; Wk^T ; Wq^T*s ; Wv^T ; bq*s] into one bf16 DRAM
    tensor xw [2256, 512]; XBAR DMA-transposes deliver both xT (d on
    partitions) and the weight block w_sb in matmul-ready layout -- no PE
    transposes, no separate weight DMAs (mixed DMACopy/transpose prologues
    get serialized by the tile scheduler's cross-DMA chaining)
  - k and q are projected in ONE matmul chain per 512-wide t-chunk
    (lhsT = [Wk | Wq*s] packed [128, 128]); the k bias is dropped (softmax
    over j is invariant to per-row shifts) and bq is added by the
    PSUM->SBUF copy (kT/qT split tiles, cross-partition-base DVE copies)
  - v is projected directly in row layout ([t, h], ap=64 matmuls) into
    v_aug [128, 16, 65] whose last column is ones so the PV matmul also
    accumulates the softmax denominator
  - attention per 512-wide i-block in S^T layout: S^T = k_tile^T @ q, exp
    on ACT (pairs of j-tiles), triangular boundary masks via affine_select
    on Pool; the upper two diagonal tiles are computed on [256:512] only
  - PV uses out[i, h] = e2_tile^T @ v_aug (ap=65), accumulating all four
    128-wide i-subtiles of a block in a single PSUM bank -> output lands
    in row layout, no output transposes
  - dummy PE matmuls at t=0 ramp the tensor engine to full clock while the
    x DMAs are in flight; exp table load is also hoisted to t=0
  - epilogue: reciprocal of the denominator column + per-subtile scale on
    DVE; bv is added on the host (softmax rows sum to 1)
"""

import sys

for _p in ("/root/.axon_site/_ro/trn_rl_repo", "/opt/trn_rl_repo"):
    if _p not in sys.path:
        sys.path.append(_p)

import numpy as np
import ml_dtypes
import concourse.bass as bass
import concourse.bacc as bacc
import concourse.tile as tile
from concourse import mybir
from concourse.bass_utils import run_bass_kernel_spmd

F32 = mybir.dt.float32
BF16 = mybir.dt.bfloat16
EXP = mybir.ActivationFunctionType.Exp

B, T, D, H = 8, 2048, 512, 64
ND = D // 128   # 4 d-chunks
NT = T // 128   # 16 j-tiles
NIB = T // 512  # 4 i-blocks
WROWS = 208     # 192 weight columns + bias row + pad to multiple of 16
N_WARM = 12     # dummy matmuls covering the ~3us PE p-state ramp


def build_body(nc, tc, ctx, dram, repeat=1):
    x_d, out_d = dram

    persist = ctx.enter_context(tc.tile_pool(name="persist", bufs=1))
    epool = ctx.enter_context(tc.tile_pool(name="epool", bufs=4))
    opool = ctx.enter_context(tc.tile_pool(name="opool", bufs=2))
    psKQ = ctx.enter_context(tc.tile_pool(name="psKQ", bufs=2, space="PSUM"))
    psV = ctx.enter_context(tc.tile_pool(name="psV", bufs=1, space="PSUM"))
    psS = ctx.enter_context(tc.tile_pool(name="psS", bufs=2, space="PSUM"))
    psO = ctx.enter_context(tc.tile_pool(name="psO", bufs=1, space="PSUM"))

    w_sb = persist.tile([128, ND, WROWS], BF16)
    xT = persist.tile([128, 4, ND, 512], BF16)  # [d%128][tchunk][d//128][t%512]
    kT = persist.tile([64, T], BF16)
    qT = persist.tile([64, T], BF16)
    v_aug = persist.tile([128, NT, 65], BF16)  # v rows + ones column
    warm_mm = persist.tile([128, 512], BF16)
    warm_out = persist.tile([128, 1], BF16)

    # hoist the exp table load to t~0 and ramp the PE to full clock with
    # dummy matmuls while the input DMAs are in flight
    nc.vector.memset(warm_mm[:], 0.0)
    nc.scalar.activation(warm_out[:], warm_mm[:, 0:1], EXP)
    for _ in range(N_WARM):
        wps = psKQ.tile([128, 512], F32, tag="kq")
        nc.tensor.matmul(wps[:], warm_mm[:, 0:128], warm_mm[:],
                         start=True, stop=True)

    # bq*scale arrives in bf16 via the weight transpose (partitions 64:128);
    # tensor_scalar needs a float32 scalar, so widen it once
    bqv = persist.tile([64, 1], F32)
    bq_ap = bqv[:]

    def proj(tch):
        tsl = slice(512 * tch, 512 * (tch + 1))
        # kq in two 256-wide halves (separate PSUM banks) so the qT/kT
        # PSUM->SBUF copies of half a overlap half b's matmuls; qT first --
        # it gates the next i-block's S matmuls
        for hf in range(2):
            hsl = slice(512 * tch + 256 * hf, 512 * tch + 256 * (hf + 1))
            kq = psKQ.tile([128, 256], F32, tag="kq")
            for dc in range(ND):
                nc.tensor.matmul(kq[:], w_sb[:, dc, 0:128],
                                 xT[:, tch, dc, 256 * hf:256 * (hf + 1)],
                                 start=(dc == 0), stop=(dc == ND - 1))
            nc.vector.tensor_scalar_add(qT[:, hsl], kq[64:128, :], bq_ap)
            nc.vector.tensor_copy(kT[:, hsl], kq[0:64, :])

        vp = psV.tile([128, 4, 64], F32, tag="v")
        for j in range(4):
            for dc in range(ND):
                nc.tensor.matmul(vp[:, j, :],
                                 xT[:, tch, dc, 128 * j:128 * (j + 1)],
                                 w_sb[:, dc, 128:192],
                                 start=(j == 0 and dc == 0),
                                 stop=(j == 3 and dc == ND - 1),
                                 skip_group_check=True)
        nc.vector.tensor_copy(v_aug[:, 4 * tch:4 * tch + 4, 0:64], vp[:])

    def attn(bi, embed=None, diag_first=False):
        i0 = 512 * bi
        o_ps = psO.tile([128, 4, 65], F32, tag="o")
        # pair descriptors: (jtA, jtB, col0, width) -- i-columns [col0, col0+w)
        full = [(2 * p, 2 * p + 1, 0, 512) for p in range(2 * bi)]
        diag = [(4 * bi, 4 * bi + 1, 0, 512),        # diagonal c=0,1
                (4 * bi + 2, 4 * bi + 3, 256, 256)]  # diagonal c=2,3
        pairs = diag + full if diag_first else full + diag
        n_pv = sum(1 for (jA, jB, _, _) in pairs for jt in (jA, jB)
                   for s in range(4) if jt - 4 * bi < 0 or s >= jt - 4 * bi)
        pv_idx = 0

        def emit_pv(e2, desc):
            nonlocal pv_idx
            jtA, jtB, col0, width = desc
            for h, jt in enumerate((jtA, jtB)):
                c = jt - 4 * bi
                for s in range(4):
                    if c >= 0 and s < c:
                        continue
                    ls = 128 * s - col0
                    nc.tensor.matmul(o_ps[:, s, :], e2[:, h, ls:ls + 128],
                                     v_aug[:, jt, :],
                                     start=(pv_idx == 0),
                                     stop=(pv_idx == n_pv - 1),
                                     skip_group_check=True)
                    pv_idx += 1

        prev = None
        for pi, desc in enumerate(pairs):
            jtA, jtB, col0, width = desc
            st = psS.tile([128, 2, 512], F32, tag="s")
            for h, jt in enumerate((jtA, jtB)):
                nc.tensor.matmul(st[:, h, 0:width],
                                 kT[:, 128 * jt:128 * (jt + 1)],
                                 qT[:, i0 + col0:i0 + col0 + width],
                                 start=True, stop=True)
            e2 = epool.tile([128, 2, 512], BF16, tag="e")
            nc.scalar.activation(e2[:, :, 0:width], st[:, :, 0:width], EXP)
            for h, jt in enumerate((jtA, jtB)):
                c = jt - 4 * bi
                if c >= 0:  # zero the strict upper triangle of the diag tile
                    lo = 128 * c - col0
                    nc.gpsimd.affine_select(
                        out=e2[:, h, lo:lo + 128], in_=e2[:, h, lo:lo + 128],
                        compare_op=mybir.AluOpType.is_ge, fill=0.0,
                        base=0, pattern=[[1, 128]], channel_multiplier=-1)
            if pi == min(1, len(pairs) - 1) and embed is not None:
                embed()  # next t-chunk projections, while ACT chews early exps
            if prev is not None:
                emit_pv(*prev)
            prev = (e2, desc)
        emit_pv(*prev)

        # store numerator+denominator raw; the host does the divide
        o_sb = opool.tile([128, 4, 65], BF16, tag="os")
        nc.vector.tensor_copy(o_sb[:], o_ps[:])
        nc.sync.dma_start(
            out_d[i0:i0 + 512, :].rearrange("(a p) h -> p a h", p=128),
            o_sb[:])

    for rep in range(repeat):
        nc.vector.memset(v_aug[:, :, 64:65], 1.0)
        # XBAR-transposed loads (out[p, c, r] = in[r, 128c + p]); keeping the
        # prologue pure-transpose on one queue avoids the tile scheduler's
        # cross-DMA serialization chains
        if rep == 0:
            nc.sync.dma_start_transpose(w_sb[:], x_d[T:T + WROWS, :])
            nc.vector.tensor_copy(bqv[:], w_sb[64:128, 0, 192:193])
        for tch in range(4):
            nc.sync.dma_start_transpose(
                xT[:, tch, :, :], x_d[512 * tch:512 * (tch + 1), :])

        proj(0)
        attn(0, embed=lambda: proj(1))
        attn(1, embed=lambda: proj(2))
        attn(2, embed=lambda: proj(3))
        attn(3, diag_first=True)


def build_nc(repeat=1):
    nc = bacc.Bacc("TRN2", target_bir_lowering=False, debug=False, num_devices=8)
    x_d = nc.dram_tensor("xw", [T + WROWS, D], BF16, kind="ExternalInput")
    out_d = nc.dram_tensor("out", [T, H + 1], BF16, kind="ExternalOutput")

    from contextlib import ExitStack
    with tile.TileContext(nc) as tc:
        with ExitStack() as ctx:
            build_body(nc, tc, ctx, (x_d, out_d), repeat=repeat)
    nc.compile()
    return nc


_NC_CACHE = {}


def _get_nc(repeat=1):
    if repeat not in _NC_CACHE:
        _NC_CACHE[repeat] = build_nc(repeat)
    return _NC_CACHE[repeat]


def make_in_maps(x, Wk, bk, Wq, bq, Wv, bv):
    scale = float(H) ** -0.5
    bf = ml_dtypes.bfloat16
    # weight block rows r, cols d: w_sb[p, dc, r] = WB[r, 128 dc + p]
    #   r 0:64 = Wk^T, 64:128 = (Wq*s)^T, 128:192 = Wv^T,
    #   r 192 cols 64:128 = bq*s (k-bias dropped: softmax shift invariance)
    WB = np.zeros((WROWS, D), np.float32)
    WB[0:64] = Wk.T
    WB[64:128] = (Wq * scale).T
    WB[128:192] = Wv.T
    WB[192, 64:128] = bq * scale
    WB = WB.astype(bf)
    return [{"xw": np.concatenate([np.asarray(x[b]).astype(bf), WB], axis=0)}
            for b in range(B)]


def kernel(x, Wk, bk, Wq, bq, Wv, bv, _repeat=1):
    x = np.asarray(x, dtype=np.float32)
    Wk = np.asarray(Wk, dtype=np.float32)
    bk = np.asarray(bk, dtype=np.float32)
    Wq = np.asarray(Wq, dtype=np.float32)
    bq = np.asarray(bq, dtype=np.float32)
    Wv = np.asarray(Wv, dtype=np.float32)
    bv = np.asarray(bv, dtype=np.float32)

    nc = _get_nc(_repeat)
    in_maps = make_in_maps(x, Wk, bk, Wq, bq, Wv, bv)
    res = run_bass_kernel_spmd(nc, in_maps, core_ids=list(range(B)))
    raw = np.stack([np.asarray(res.results[b]["out"]).astype(np.float32)
                    for b in range(B)], axis=0)
    return raw[..., 0:64] / raw[..., 64:65] + bv[None, None, :]
